# revision 27
# baseline (speedup 1.0000x reference)
"""Multi-head attention Trainium2 kernel (B=4, T=2048, C=1024, H=16, D=64).

Sharding: 8 cores = 4 batches x 2 head-groups (data parallel on B, tensor
parallel on H). Each core computes attention for 1 batch and 8 heads plus the
partial out-projection for its head rows; the host sums the two partials per
batch (the out-proj "all-reduce"); bias is applied on-device by hg=0 cores.

Device layout notes (per core):
  xT  [C, T]   bf16  x[b] transposed on host
  wq/wk/wv [C, 512] bf16 per-head-group column slices of w_qkv
  wo  [512, C] bf16  row slice of w_out
  bias [1, C]  f32   b_out on hg=0 cores, zeros on hg=1
  y   [T, C]   f32   partial output

  QT/KT: [D,T] per head, two heads packed per 128-partition tile. Scores
  S^T[k,q] matmuls alternate the two heads (disjoint PE row groups) so
  consecutive matmuls can overlap in the array. exp() runs on ScalarE
  straight out of PSUM (logits bounded, no max subtraction needed) into an
  interleaved expS ring in SBUF. V is kept natural [T,D] with an appended
  ones column so the M=65 PV matmul produces O^T (rows 0..63) and the
  softmax denominators (row 64) in one pass. Reciprocal via fast DVE approx
  (input must sit at partition 0), partition-broadcast on GpSimd, then the
  out-projection consumes Theta^T directly as the stationary operand.
"""

import numpy as np
import ml_dtypes

import concourse.bacc as bacc
import concourse.mybir as mybir
import concourse.tile as tile
from concourse.bass_utils import run_bass_kernel_spmd

# ---- custom DVE exp: exp(16m) = (deg-4 poly(m))^16, two DVE passes ----
# scores arrive pre-scaled by 0.125/16 (folded into wq on host), so the
# ACT path uses activation scale=16 and the DVE path consumes m directly.
import concourse.dve_ops as _DO
from concourse.dve_spec import (Spec as _Spec, Src0 as _Src0, C0 as _C0,
                                C1 as _C1, C2 as _C2, One as _One, sq as _sq,
                                lower as _lower, _spill_c3_to_src1, _has_src1)
from concourse.dve_uop import DveOpSpec as _DveOpSpec

# minimax deg-4 for exp(m) on |m| <= 0.375; (poly)^16 rel err ~1e-4
_EXPC = (0.999976373245791, 0.5000807312260734,
         0.16791041384056227, 0.041095847608236376)


def _register_dve(name, spec):
    if name in _DO._SUB_OPCODE_FOR_NAME:
        return next(op for op in _DO.OPS if op.name == name)
    row = _DO._CUSTOM_DVE_ROW_BASE + len(_DO.OPS)
    shas = {}
    for ver in ("v3", "v4"):
        shas[ver] = _DveOpSpec(name=name, opcode=row, uops=_lower(spec, ver=ver),
                               rd1_en=_has_src1(spec)).sha(ver)
    op = _DO.DveOp(name, spec, subdim=False, uops_sha=shas)
    _DO.OPS.append(op)
    _DO._SUB_OPCODE_FOR_NAME[name] = row
    _DO.CUSTOM_DVE_SPECS[name] = spec
    return op


_EXPP4 = _register_dve("EXPP4_ANT", _Spec(
    body=_spill_c3_to_src1(
        _One + _Src0 * (_C0 + _Src0 * (_C1 + _Src0 * (_C2 + _Src0 * _DO.C3)))),
    reference=lambda in0, in1, s0, s1, imm2:
        1.0 + in0 * (s0 + in0 * (s1 + in0 * (imm2 + in0 * in1))),
))
_SQ4 = _register_dve("SQ4_ANT", _Spec(
    body=_sq(_sq(_sq(_sq(_Src0)))),
    reference=lambda in0, in1, s0, s1, imm2: in0 ** 16,
))

B, T, C, H, D = 4, 2048, 1024, 16, 64
HPC = 8          # heads per core
PAIRS = HPC // 2
CT = C // 128    # 8 contraction tiles for projections
TT = T // 128    # 16 t-tiles (also k-tiles of attention)
QC = T // 512    # 4 query chunks
JC = C // 512    # 2 out-proj column chunks
BF16 = mybir.dt.bfloat16
F32 = mybir.dt.float32
EXP = mybir.ActivationFunctionType.Exp
DVE_KTS = ()   # exp on DVE regressed: extra SBUF/PSUM traffic dilates everything

_CACHED_NC = None


def _emit(nc, tc, xT_d, wq_d, wk_d, wv_d, wo_d, bias_d, y_d):
    import contextlib
    with contextlib.ExitStack() as ctx:
        persist = ctx.enter_context(tc.tile_pool(name="persist", bufs=1))
        work = ctx.enter_context(tc.tile_pool(name="work", bufs=2))
        spsum = ctx.enter_context(tc.tile_pool(name="spsum", bufs=2, space="PSUM"))
        apsum = ctx.enter_context(tc.tile_pool(name="apsum", bufs=2, space="PSUM"))
        ppsum = ctx.enter_context(tc.tile_pool(name="ppsum", bufs=2, space="PSUM"))

        # ---- static loads ----
        # DMA descriptor issue costs ~0.6us on the issuing engine, so spread
        # across four engines round-robin, most-critical transfers first:
        # wk + xT first-quarters feed the K projection that gates the first
        # scores/exp; wq next; V/out-proj weights and xT tails can trail.
        # sync also brokers semaphores: keep it to 1 descriptor in 5 so the
        # first scores->exp handoff isn't queued behind DMA issue
        engs = [nc.scalar, nc.gpsimd, nc.scalar, nc.gpsimd, nc.sync]
        ei = [0]

        def dma(out, in_):
            engs[ei[0] % len(engs)].dma_start(out=out, in_=in_)
            ei[0] += 1

        # HAM warmup: ~10 matmuls on a zeroed tile keep the PE active from
        # t~0 so the real projection chains run at 2.4GHz, not cold 1.2
        warm_sb = persist.tile([128, 512], BF16, tag="warm", name="warm")
        nc.vector.memset(warm_sb, 0.0)
        wps = ppsum.tile([128, 512], F32, tag="proj", name="warmps")
        for _ in range(10):
            nc.tensor.matmul(wps, lhsT=warm_sb[:, 0:128], rhs=warm_sb,
                             start=True, stop=True)

        xT_sb = [persist.tile([128, T], BF16, tag=f"xT{i}", name=f"xT{i}")
                 for i in range(CT)]
        w_sb = {}
        for wname, wd in (("wk", wk_d), ("wq", wq_d), ("wv", wv_d)):
            for i in range(CT):
                t = persist.tile([128, 512], BF16, tag=f"{wname}{i}", name=f"{wname}{i}")
                w_sb[(wname, i)] = t
        for i in range(CT):
            dma(w_sb[("wk", i)], wk_d[i * 128:(i + 1) * 128, :])
        for i in range(CT):
            dma(xT_sb[i][:, 0:512], xT_d[i * 128:(i + 1) * 128, 0:512])
        for i in range(CT):
            dma(w_sb[("wq", i)], wq_d[i * 128:(i + 1) * 128, :])
        for i in range(CT):
            dma(xT_sb[i][:, 512:1024], xT_d[i * 128:(i + 1) * 128, 512:1024])
        for i in range(CT):
            dma(w_sb[("wv", i)], wv_d[i * 128:(i + 1) * 128, :])
        for i in range(CT):
            dma(xT_sb[i][:, 1024:T], xT_d[i * 128:(i + 1) * 128, 1024:T])
        wo_sb = []
        for i in range(4):
            t = persist.tile([128, C], BF16, tag=f"wo{i}", name=f"wo{i}")
            dma(t, wo_d[i * 128:(i + 1) * 128, :])
            wo_sb.append(t)
        bias_sb = persist.tile([1, C], F32, tag="bias", name="bias")
        dma(bias_sb, bias_d[0:1, :])
        bias_bc = persist.tile([128, C], F32, tag="bias_bc", name="bias_bc")
        nc.gpsimd.partition_broadcast(bias_bc, bias_sb)
        c4_sb = persist.tile([128, 1], F32, tag="c4", name="c4")
        nc.vector.memset(c4_sb, _EXPC[3])

        # V natural [T, 512] + ones column per head -> Vaug tiles [128, 8, 65]
        vaug = [persist.tile([128, HPC, D + 1], BF16, tag=f"vaug{tt}", name=f"vaug{tt}")
                for tt in range(TT)]

        def v_chunk(tt):
            vt = vaug[tt]
            # alternate pools: apsum is idle until the first PV chain starts,
            # so V gets 4 accumulator banks during the startup weave
            pool, tg = (ppsum, "proj") if tt % 2 == 0 else (apsum, "acc")
            ps = pool.tile([128, 512], F32, tag=tg, name="vps")
            for c in range(CT):
                nc.tensor.matmul(ps, lhsT=xT_sb[c][:, tt * 128:(tt + 1) * 128],
                                 rhs=w_sb[("wv", c)], start=(c == 0), stop=(c == CT - 1))
            nc.vector.tensor_copy(
                out=vt[:, :, 0:D],
                in_=ps.rearrange("p (h d) -> p h d", h=HPC))
            nc.vector.memset(vt[:, :, D:D + 1], 1.0)

        # Q^T / K^T tiles [128 = 2 heads x 64, T]; filled lazily per pair so
        # later pairs' projections overlap earlier pairs' ACT-bound attention
        # 2-slot rotation: pair p+2's projection reuses pair p's slot (dead
        # after pair p's last section, which precedes those filler writes)
        qt_sb = [persist.tile([128, T], BF16, tag="qt", bufs=2, name=f"qt{p}")
                 for p in range(PAIRS)]
        kt_sb = [persist.tile([128, T], BF16, tag="kt", bufs=2, name=f"kt{p}")
                 for p in range(PAIRS)]

        def project_chunk(p, dst, wname, qc):
            ps = ppsum.tile([128, 512], F32, tag="proj", name="qkps")
            for c in range(CT):
                nc.tensor.matmul(
                    ps,
                    lhsT=w_sb[(wname, c)][:, p * 128:(p + 1) * 128],
                    rhs=xT_sb[c][:, qc * 512:(qc + 1) * 512],
                    start=(c == 0), stop=(c == CT - 1))
            nc.vector.tensor_copy(out=dst[:, qc * 512:(qc + 1) * 512], in_=ps)

        # pair-0: only the first K/Q chunks upfront so the first scores/exp
        # start as soon as wk + the first xT quarter land; K1-3 and Q1-3 are
        # woven into (p0, qc0) as per-kt fillers just ahead of their readers
        project_chunk(0, kt_sb[0], "wk", 0)
        project_chunk(0, qt_sb[0], "wq", 0)
        p0_fillers = {
            0: [("wk", 1)], 1: [("wk", 2)], 2: [("wk", 3)],
            4: [("wq", 1)], 6: [("wq", 2)], 8: [("wq", 3)],
        }

        # ---- attention ----
        # expS ring: interleaved [h0 kt | h1 kt] units of 512, RING=40 units
        # (1.25 sections) so exp of section s+1 can run ahead while PV of
        # section s drains; subtile deps handle the wrap-around reuse.
        RING = 46
        exps = persist.tile([128, RING * 512], BF16, tag="expS", name="expS")
        tht_sb = [persist.tile([128, T], BF16, tag=f"tht{p}", name=f"tht{p}")
                  for p in range(PAIRS)]
        # filler work emitted after each (p, qc) section: the next pair's
        # projections (and, for p0/qc0, the V projection) fill PE bubbles
        # while the current attention chunk is ACT-paced
        # just-in-time projection fillers: each entry (pair, wname, chunk) is
        # emitted after section (p, qc); K chunks precede Q chunks since
        # scores(p, qc0) read all of K^T but only one Q^T chunk
        fillers = {
            (0, 1): [(1, "wk", 0), (1, "wk", 1), (1, "wk", 2)],
            (0, 2): [(1, "wk", 3), (1, "wq", 0), (1, "wq", 1)],
            (0, 3): [(1, "wq", 2), (1, "wq", 3)],
            (1, 0): [(2, "wk", 0), (2, "wk", 1)],
            (1, 1): [(2, "wk", 2), (2, "wk", 3)],
            (1, 2): [(2, "wq", 0), (2, "wq", 1)],
            (1, 3): [(2, "wq", 2), (2, "wq", 3)],
            (2, 0): [(3, "wk", 0), (3, "wk", 1)],
            (2, 1): [(3, "wk", 2), (3, "wk", 3)],
            (2, 2): [(3, "wq", 0), (3, "wq", 1)],
            (2, 3): [(3, "wq", 2), (3, "wq", 3)],
        }

        def out_proj_group(tt):
            ysb = work.tile([128, C], F32, tag="ysb", bufs=3, name="ysb")
            for jc in range(JC):
                jsl = slice(jc * 512, (jc + 1) * 512)
                # alternate accumulator pools: ppsum is mostly idle during
                # the last pair (few projection fillers left)
                pool, tg = ((apsum, "acc") if (tt + jc) % 2 == 0
                            else (ppsum, "proj"))
                yps = pool.tile([128, 512], F32, tag=tg, name="yps")
                for pp in range(PAIRS):
                    nc.tensor.matmul(
                        yps, lhsT=tht_sb[pp][:, tt * 128:(tt + 1) * 128],
                        rhs=wo_sb[pp][:, jsl],
                        start=(pp == 0), stop=(pp == PAIRS - 1))
                nc.vector.tensor_add(out=ysb[:, jsl], in0=yps,
                                     in1=bias_bc[:, jsl])
            eng = nc.sync if tt % 2 == 0 else nc.gpsimd
            eng.dma_start(out=y_d[tt * 128:(tt + 1) * 128, :], in_=ysb)

        # final 4 out-proj groups split so only the pair-3 contraction sits
        # in the serial tail: pairs 0-2 accumulate into SBUF during the last
        # section's exp-paced slack, pair 3 adds in after tht[3] lands
        yfin = {}

        def out_proj_early(tt):
            ysb = work.tile([128, C], F32, tag="ysbf", bufs=4, name="ysbf")
            for jc in range(JC):
                jsl = slice(jc * 512, (jc + 1) * 512)
                pool, tg = ((apsum, "acc") if (tt + jc) % 2 == 0
                            else (ppsum, "proj"))
                yps = pool.tile([128, 512], F32, tag=tg, name="yps")
                for pp in range(PAIRS - 1):
                    nc.tensor.matmul(
                        yps, lhsT=tht_sb[pp][:, tt * 128:(tt + 1) * 128],
                        rhs=wo_sb[pp][:, jsl],
                        start=(pp == 0), stop=(pp == PAIRS - 2))
                nc.vector.tensor_add(out=ysb[:, jsl], in0=yps,
                                     in1=bias_bc[:, jsl])
            yfin[tt] = ysb

        def out_proj_late(tt):
            ysb = yfin[tt]
            for jc in range(JC):
                jsl = slice(jc * 512, (jc + 1) * 512)
                pool, tg = ((ppsum, "proj") if (tt + jc) % 2 == 0
                            else (apsum, "acc"))
                yps = pool.tile([128, 512], F32, tag=tg, name="yps")
                nc.tensor.matmul(
                    yps, lhsT=tht_sb[PAIRS - 1][:, tt * 128:(tt + 1) * 128],
                    rhs=wo_sb[PAIRS - 1][:, jsl], start=True, stop=True)
                nc.vector.tensor_add(out=ysb[:, jsl], in0=yps, in1=ysb[:, jsl])
            # tail DMAs: split each row-block across engines/queues so the
            # final 2MB ships in parallel instead of serializing on one ring
            dengs = [nc.sync, nc.scalar, nc.gpsimd]
            for jc in range(JC):
                jsl = slice(jc * 512, (jc + 1) * 512)
                dengs[(2 * tt + jc) % 3].dma_start(
                    out=y_d[tt * 128:(tt + 1) * 128, jsl], in_=ysb[:, jsl])

        ring_base = 0
        for p in range(PAIRS):
            for qc in range(QC):
                qsl = slice(qc * 512, (qc + 1) * 512)

                def unit(kt, lh):
                    u = (ring_base + 2 * kt + lh) % RING
                    return slice(u * 512, (u + 1) * 512)

                # scores + exp: adjacent matmuls alternate PE row groups
                # (h0 rows 0-63, h1 rows 64-127) so they can overlap
                for kt in range(TT):
                    ps = spsum.tile([128, 1024], F32, tag="mm", name="sps")
                    for lh in range(2):
                        hsl = slice(lh * 64, (lh + 1) * 64)
                        nc.tensor.matmul(
                            ps[:, lh * 512:(lh + 1) * 512],
                            lhsT=kt_sb[p][hsl, kt * 128:(kt + 1) * 128],
                            rhs=qt_sb[p][hsl, qsl],
                            start=True, stop=True)
                    u0 = (ring_base + 2 * kt) % RING
                    if kt in DVE_KTS:
                        # exp on DVE: deg-4 poly then ^16, two custom passes
                        mid = work.tile([128, 1024], F32, tag="escr", name="escr")
                        nc.vector._custom_dve(
                            _EXPP4, out=mid, in0=ps, in1=c4_sb,
                            s0=_EXPC[0], s1=_EXPC[1], imm2=_EXPC[2])
                        nc.vector._custom_dve(
                            _SQ4, out=exps[:, u0 * 512:(u0 + 2) * 512], in0=mid)
                    else:
                        nc.scalar.activation(
                            out=exps[:, u0 * 512:(u0 + 2) * 512],
                            in_=ps, func=EXP, scale=16.0)
                    if p == 0 and qc == 0:
                        # V projection + remaining pair-0 K/Q chunks woven
                        # into the exp-paced score loop
                        v_chunk(kt)
                        for wname, fqc in p0_fillers.get(kt, []):
                            dst = qt_sb[0] if wname == "wq" else kt_sb[0]
                            project_chunk(0, dst, wname, fqc)
                    # spread the previous chunk's out-projection through the
                    # score loop (one group per 4 kts) instead of a burst
                    if p == PAIRS - 1 and qc >= 1 and kt % 4 == 3:
                        out_proj_group(4 * (qc - 1) + kt // 4)
                # the final groups' pair 0-2 contraction runs during the last
                # section's slack (out_proj_early); pair 3 lands in the tail
                if p == PAIRS - 1 and qc == QC - 1:
                    for tt in range(4 * (QC - 1), 4 * QC):
                        out_proj_early(tt)
                # PV: both heads' accumulation chains interleaved so ring
                # units free in kt order and exp of the next section can
                # overwrite them while these chains drain. In the very last
                # section the heads run sequentially instead, so h0's
                # normalize chain overlaps h1's PV stream (shorter tail).
                ops = [apsum.tile([D + 1, 512], F32, tag="acc", name=f"ops{lh}")
                       for lh in range(2)]
                def normalize(lh):
                    # copy sums to partition 0 first: the custom-DVE fast
                    # reciprocal misreads partition-shifted inputs
                    ssb = work.tile([1, 512], F32, tag="ssb", name="ssb")
                    nc.vector.tensor_copy(out=ssb, in_=ops[lh][D:D + 1, :])
                    rsb = work.tile([1, 512], F32, tag="rsb", name="rsb")
                    nc.vector.reciprocal_approx_fast(out=rsb, in_=ssb)
                    rbc = work.tile([64, 512], F32, tag="rbc", name="rbc")
                    nc.gpsimd.partition_broadcast(rbc, rsb)
                    nc.vector.tensor_mul(
                        out=tht_sb[p][lh * 64:(lh + 1) * 64, qsl],
                        in0=ops[lh][0:D, :], in1=rbc)

                for lh in range(2):
                    for kt in range(TT):
                        nc.tensor.matmul(
                            ops[lh], lhsT=vaug[kt][:, 2 * p + lh, :],
                            rhs=exps[:, unit(kt, lh)],
                            start=(kt == 0), stop=(kt == TT - 1))
                    normalize(lh)
                ring_base = (ring_base + 2 * TT) % RING
                for fp, wname, fqc in fillers.get((p, qc), []):
                    dst = qt_sb[fp] if wname == "wq" else kt_sb[fp]
                    project_chunk(fp, dst, wname, fqc)

        for tt in range(4 * (QC - 1), 4 * QC):
            out_proj_late(tt)


def _build():
    nc = bacc.Bacc("TRN2", target_bir_lowering=False)
    xT_d = nc.dram_tensor("xT", [C, T], BF16, kind="ExternalInput")
    wq_d = nc.dram_tensor("wq", [C, 512], BF16, kind="ExternalInput")
    wk_d = nc.dram_tensor("wk", [C, 512], BF16, kind="ExternalInput")
    wv_d = nc.dram_tensor("wv", [C, 512], BF16, kind="ExternalInput")
    wo_d = nc.dram_tensor("wo", [512, C], BF16, kind="ExternalInput")
    bias_d = nc.dram_tensor("bias", [1, C], F32, kind="ExternalInput")
    y_d = nc.dram_tensor("y", [T, C], F32, kind="ExternalOutput")
    with tile.TileContext(nc) as tc:
        _emit(nc, tc, xT_d, wq_d, wk_d, wv_d, wo_d, bias_d, y_d)
    if not nc.is_finalized():
        nc.finalize()
    return nc


def get_nc():
    global _CACHED_NC
    if _CACHED_NC is None:
        _CACHED_NC = _build()
    return _CACHED_NC


def make_in_maps(x, w_qkv, w_out, b_out):
    bf = ml_dtypes.bfloat16
    x = np.asarray(x, dtype=np.float32)
    w_qkv = np.asarray(w_qkv, dtype=np.float32)
    w_out = np.asarray(w_out, dtype=np.float32)
    b_out = np.asarray(b_out, dtype=np.float32)
    in_maps = []
    for core in range(8):
        b, hg = core // 2, core % 2
        cs = slice(hg * 512, (hg + 1) * 512)
        bias = b_out if hg == 0 else np.zeros_like(b_out)
        in_maps.append({
            "xT": np.ascontiguousarray(x[b].T).astype(bf),
            # wq pre-scaled by the attention scale (1/8) / 16 so scores psum
            # holds m with exp(s/8) = exp(16m); 2^-7 is exact in bf16
            "wq": np.ascontiguousarray(w_qkv[:, 0 * C:][:, cs] * 0.0078125).astype(bf),
            "wk": np.ascontiguousarray(w_qkv[:, 1 * C:][:, cs]).astype(bf),
            "wv": np.ascontiguousarray(w_qkv[:, 2 * C:][:, cs]).astype(bf),
            "wo": np.ascontiguousarray(w_out[cs, :]).astype(bf),
            "bias": np.ascontiguousarray(bias.reshape(1, C), dtype=np.float32),
        })
    return in_maps


def _ensure_ntff_hook():
    """Register the axon NTFF profile hook if the container's antenv lacks
    axon_hooks (test/profiling use only; never needed for plain kernel())."""
    import sys
    import types
    try:
        from antenv import axon_hooks  # noqa: F401
    except ImportError:
        mod = types.ModuleType("antenv.axon_hooks")
        mod._hook = None

        def set_axon_ntff_profile_hook(hook, _m=mod):
            _m._hook = hook

        def get_axon_ntff_profile_hook(_m=mod):
            return _m._hook

        mod.set_axon_ntff_profile_hook = set_axon_ntff_profile_hook
        mod.get_axon_ntff_profile_hook = get_axon_ntff_profile_hook
        sys.modules["antenv.axon_hooks"] = mod
        import antenv
        antenv.axon_hooks = mod
    import antenv.axon_hooks as ah
    if ah.get_axon_ntff_profile_hook() is None:
        from trn_agent_boot.trn_boot import _ntff_profile_via_ctypes
        ah.set_axon_ntff_profile_hook(
            _ntff_profile_via_ctypes("/opt/axon/libaxon_pjrt.so"))


def kernel(x, w_qkv, w_out, b_out, _trace=False, _trace_kwargs=None):
    nc = get_nc()
    in_maps = make_in_maps(x, w_qkv, w_out, b_out)
    kwargs = {}
    if _trace:
        try:
            _ensure_ntff_hook()
        except Exception as e:
            print(f"NTFF hook setup failed ({e}); running without trace")
        else:
            kwargs.update(trace=True, **(_trace_kwargs or {}))
    res = run_bass_kernel_spmd(nc, in_maps, core_ids=list(range(8)), **kwargs)
    out = np.empty((B, T, C), dtype=np.float32)
    for b in range(B):
        out[b] = res.results[2 * b]["y"] + res.results[2 * b + 1]["y"]
    if _trace:
        return out, res
    return out



# revision 30
# speedup vs baseline: 1.0172x; 1.0172x over previous
"""Multi-head attention Trainium2 kernel (B=4, T=2048, C=1024, H=16, D=64).

Sharding: 8 cores = 4 batches x 2 head-groups (data parallel on B, tensor
parallel on H). Each core computes attention for 1 batch and 8 heads plus the
partial out-projection for its head rows; the host sums the two partials per
batch (the out-proj "all-reduce"); bias is applied on-device by hg=0 cores.

Device layout notes (per core):
  xT  [C, T]   bf16  x[b] transposed on host
  wq/wk/wv [C, 512] bf16 per-head-group column slices of w_qkv
  wo  [512, C] bf16  row slice of w_out
  bias [1, C]  f32   b_out on hg=0 cores, zeros on hg=1
  y   [T, C]   f32   partial output

  QT/KT: [D,T] per head, two heads packed per 128-partition tile. Scores
  S^T[k,q] matmuls alternate the two heads (disjoint PE row groups) so
  consecutive matmuls can overlap in the array. exp() runs on ScalarE
  straight out of PSUM (logits bounded, no max subtraction needed) into an
  interleaved expS ring in SBUF. V is kept natural [T,D] with an appended
  ones column so the M=65 PV matmul produces O^T (rows 0..63) and the
  softmax denominators (row 64) in one pass. Reciprocal via fast DVE approx
  (input must sit at partition 0), partition-broadcast on GpSimd, then the
  out-projection consumes Theta^T directly as the stationary operand.
"""

import numpy as np
import ml_dtypes

import concourse.bacc as bacc
import concourse.mybir as mybir
import concourse.tile as tile
from concourse.bass_utils import run_bass_kernel_spmd

# ---- custom DVE exp: exp(16m) = (deg-4 poly(m))^16, two DVE passes ----
# scores arrive pre-scaled by 0.125/16 (folded into wq on host), so the
# ACT path uses activation scale=16 and the DVE path consumes m directly.
import concourse.dve_ops as _DO
from concourse.dve_spec import (Spec as _Spec, Src0 as _Src0, C0 as _C0,
                                C1 as _C1, C2 as _C2, One as _One, sq as _sq,
                                lower as _lower, _spill_c3_to_src1, _has_src1)
from concourse.dve_uop import DveOpSpec as _DveOpSpec

# minimax deg-4 for exp(m) on |m| <= 0.375; (poly)^16 rel err ~1e-4
_EXPC = (0.999976373245791, 0.5000807312260734,
         0.16791041384056227, 0.041095847608236376)


def _register_dve(name, spec):
    if name in _DO._SUB_OPCODE_FOR_NAME:
        return next(op for op in _DO.OPS if op.name == name)
    row = _DO._CUSTOM_DVE_ROW_BASE + len(_DO.OPS)
    shas = {}
    for ver in ("v3", "v4"):
        shas[ver] = _DveOpSpec(name=name, opcode=row, uops=_lower(spec, ver=ver),
                               rd1_en=_has_src1(spec)).sha(ver)
    op = _DO.DveOp(name, spec, subdim=False, uops_sha=shas)
    _DO.OPS.append(op)
    _DO._SUB_OPCODE_FOR_NAME[name] = row
    _DO.CUSTOM_DVE_SPECS[name] = spec
    return op


_EXPP4 = _register_dve("EXPP4_ANT", _Spec(
    body=_spill_c3_to_src1(
        _One + _Src0 * (_C0 + _Src0 * (_C1 + _Src0 * (_C2 + _Src0 * _DO.C3)))),
    reference=lambda in0, in1, s0, s1, imm2:
        1.0 + in0 * (s0 + in0 * (s1 + in0 * (imm2 + in0 * in1))),
))
_SQ4 = _register_dve("SQ4_ANT", _Spec(
    body=_sq(_sq(_sq(_sq(_Src0)))),
    reference=lambda in0, in1, s0, s1, imm2: in0 ** 16,
))

B, T, C, H, D = 4, 2048, 1024, 16, 64
HPC = 8          # heads per core
PAIRS = HPC // 2
CT = C // 128    # 8 contraction tiles for projections
TT = T // 128    # 16 t-tiles (also k-tiles of attention)
QC = T // 512    # 4 query chunks
JC = C // 512    # 2 out-proj column chunks
BF16 = mybir.dt.bfloat16
F32 = mybir.dt.float32
EXP = mybir.ActivationFunctionType.Exp
DVE_KTS = ()   # exp on DVE regressed: extra SBUF/PSUM traffic dilates everything

_CACHED_NC = None


def _emit(nc, tc, xT_d, wq_d, wk_d, wv_d, wo_d, bias_d, y_d):
    import contextlib
    with contextlib.ExitStack() as ctx:
        persist = ctx.enter_context(tc.tile_pool(name="persist", bufs=1))
        work = ctx.enter_context(tc.tile_pool(name="work", bufs=2))
        spsum = ctx.enter_context(tc.tile_pool(name="spsum", bufs=2, space="PSUM"))
        apsum = ctx.enter_context(tc.tile_pool(name="apsum", bufs=2, space="PSUM"))
        ppsum = ctx.enter_context(tc.tile_pool(name="ppsum", bufs=2, space="PSUM"))

        # ---- static loads ----
        # DMA descriptor issue costs ~0.6us on the issuing engine, so spread
        # across four engines round-robin, most-critical transfers first:
        # wk + xT first-quarters feed the K projection that gates the first
        # scores/exp; wq next; V/out-proj weights and xT tails can trail.
        # gpsimd-heavy: scalar must stay clear for the first EXP and sync
        # brokers semaphores, so they only help with the critical first wave
        engs = [nc.gpsimd, nc.sync, nc.gpsimd, nc.gpsimd]
        ei = [0]

        def dma(out, in_):
            engs[ei[0] % len(engs)].dma_start(out=out, in_=in_)
            ei[0] += 1

        # HAM warmup: ~10 matmuls on a zeroed tile keep the PE active from
        # t~0 so the real projection chains run at 2.4GHz, not cold 1.2
        warm_sb = persist.tile([128, 512], BF16, tag="warm", name="warm")
        nc.vector.memset(warm_sb, 0.0)
        wps = ppsum.tile([128, 512], F32, tag="proj", name="warmps")
        for _ in range(10):
            nc.tensor.matmul(wps, lhsT=warm_sb[:, 0:128], rhs=warm_sb,
                             start=True, stop=True)

        xT_sb = [persist.tile([128, T], BF16, tag=f"xT{i}", name=f"xT{i}")
                 for i in range(CT)]
        w_sb = {}
        for wname, wd in (("wk", wk_d), ("wq", wq_d), ("wv", wv_d)):
            for i in range(CT):
                t = persist.tile([128, 512], BF16, tag=f"{wname}{i}", name=f"{wname}{i}")
                w_sb[(wname, i)] = t
        for i in range(CT):
            dma(w_sb[("wk", i)], wk_d[i * 128:(i + 1) * 128, :])
        for i in range(CT):
            dma(xT_sb[i][:, 0:512], xT_d[i * 128:(i + 1) * 128, 0:512])
        for i in range(CT):
            dma(w_sb[("wq", i)], wq_d[i * 128:(i + 1) * 128, :])
        for i in range(CT):
            dma(xT_sb[i][:, 512:1024], xT_d[i * 128:(i + 1) * 128, 512:1024])
        for i in range(CT):
            dma(w_sb[("wv", i)], wv_d[i * 128:(i + 1) * 128, :])
        for i in range(CT):
            dma(xT_sb[i][:, 1024:T], xT_d[i * 128:(i + 1) * 128, 1024:T])
        wo_sb = []
        for i in range(4):
            t = persist.tile([128, C], BF16, tag=f"wo{i}", name=f"wo{i}")
            dma(t, wo_d[i * 128:(i + 1) * 128, :])
            wo_sb.append(t)
        bias_sb = persist.tile([1, C], F32, tag="bias", name="bias")
        dma(bias_sb, bias_d[0:1, :])
        bias_bc = persist.tile([128, C], F32, tag="bias_bc", name="bias_bc")
        nc.gpsimd.partition_broadcast(bias_bc, bias_sb)
        c4_sb = persist.tile([128, 1], F32, tag="c4", name="c4")
        nc.vector.memset(c4_sb, _EXPC[3])

        # V natural [T, 512] + ones column per head -> Vaug tiles [128, 8, 65]
        vaug = [persist.tile([128, HPC, D + 1], BF16, tag=f"vaug{tt}", name=f"vaug{tt}")
                for tt in range(TT)]

        def v_chunk(tt):
            vt = vaug[tt]
            # alternate pools: apsum is idle until the first PV chain starts,
            # so V gets 4 accumulator banks during the startup weave
            pool, tg = (ppsum, "proj") if tt % 2 == 0 else (apsum, "acc")
            ps = pool.tile([128, 512], F32, tag=tg, name="vps")
            for c in range(CT):
                nc.tensor.matmul(ps, lhsT=xT_sb[c][:, tt * 128:(tt + 1) * 128],
                                 rhs=w_sb[("wv", c)], start=(c == 0), stop=(c == CT - 1))
            nc.vector.tensor_copy(
                out=vt[:, :, 0:D],
                in_=ps.rearrange("p (h d) -> p h d", h=HPC))
            nc.vector.memset(vt[:, :, D:D + 1], 1.0)

        # Q^T / K^T tiles [128 = 2 heads x 64, T]; filled lazily per pair so
        # later pairs' projections overlap earlier pairs' ACT-bound attention
        # 2-slot rotation: pair p+2's projection reuses pair p's slot (dead
        # after pair p's last section, which precedes those filler writes)
        qt_sb = [persist.tile([128, T], BF16, tag="qt", bufs=2, name=f"qt{p}")
                 for p in range(PAIRS)]
        kt_sb = [persist.tile([128, T], BF16, tag="kt", bufs=2, name=f"kt{p}")
                 for p in range(PAIRS)]

        def project_chunk(p, dst, wname, qc):
            ps = ppsum.tile([128, 512], F32, tag="proj", name="qkps")
            for c in range(CT):
                nc.tensor.matmul(
                    ps,
                    lhsT=w_sb[(wname, c)][:, p * 128:(p + 1) * 128],
                    rhs=xT_sb[c][:, qc * 512:(qc + 1) * 512],
                    start=(c == 0), stop=(c == CT - 1))
            nc.vector.tensor_copy(out=dst[:, qc * 512:(qc + 1) * 512], in_=ps)

        # pair-0: only the first K/Q chunks upfront so the first scores/exp
        # start as soon as wk + the first xT quarter land; K1-3 and Q1-3 are
        # woven into (p0, qc0) as per-kt fillers just ahead of their readers
        project_chunk(0, kt_sb[0], "wk", 0)
        project_chunk(0, qt_sb[0], "wq", 0)
        p0_fillers = {
            0: [("wk", 1)], 1: [("wk", 2)], 2: [("wk", 3)],
            4: [("wq", 1)], 6: [("wq", 2)], 8: [("wq", 3)],
        }

        # ---- attention ----
        # expS ring: interleaved [h0 kt | h1 kt] units of 512, RING=40 units
        # (1.25 sections) so exp of section s+1 can run ahead while PV of
        # section s drains; subtile deps handle the wrap-around reuse.
        RING = 46
        exps = persist.tile([128, RING * 512], BF16, tag="expS", name="expS")
        tht_sb = [persist.tile([128, T], BF16, tag=f"tht{p}", name=f"tht{p}")
                  for p in range(PAIRS)]
        # filler work emitted after each (p, qc) section: the next pair's
        # projections (and, for p0/qc0, the V projection) fill PE bubbles
        # while the current attention chunk is ACT-paced
        # just-in-time projection fillers: each entry (pair, wname, chunk) is
        # emitted after section (p, qc); K chunks precede Q chunks since
        # scores(p, qc0) read all of K^T but only one Q^T chunk
        fillers = {
            (0, 1): [(1, "wk", 0), (1, "wk", 1), (1, "wk", 2)],
            (0, 2): [(1, "wk", 3), (1, "wq", 0), (1, "wq", 1)],
            (0, 3): [(1, "wq", 2), (1, "wq", 3)],
            (1, 0): [(2, "wk", 0), (2, "wk", 1)],
            (1, 1): [(2, "wk", 2), (2, "wk", 3)],
            (1, 2): [(2, "wq", 0), (2, "wq", 1)],
            (1, 3): [(2, "wq", 2), (2, "wq", 3)],
            (2, 0): [(3, "wk", 0), (3, "wk", 1)],
            (2, 1): [(3, "wk", 2), (3, "wk", 3)],
            (2, 2): [(3, "wq", 0), (3, "wq", 1)],
            (2, 3): [(3, "wq", 2), (3, "wq", 3)],
        }

        def out_proj_group(tt):
            ysb = work.tile([128, C], F32, tag="ysb", bufs=3, name="ysb")
            for jc in range(JC):
                jsl = slice(jc * 512, (jc + 1) * 512)
                # alternate accumulator pools: ppsum is mostly idle during
                # the last pair (few projection fillers left)
                pool, tg = ((apsum, "acc") if (tt + jc) % 2 == 0
                            else (ppsum, "proj"))
                yps = pool.tile([128, 512], F32, tag=tg, name="yps")
                for pp in range(PAIRS):
                    nc.tensor.matmul(
                        yps, lhsT=tht_sb[pp][:, tt * 128:(tt + 1) * 128],
                        rhs=wo_sb[pp][:, jsl],
                        start=(pp == 0), stop=(pp == PAIRS - 1))
                nc.vector.tensor_add(out=ysb[:, jsl], in0=yps,
                                     in1=bias_bc[:, jsl])
            eng = nc.sync if tt % 2 == 0 else nc.gpsimd
            eng.dma_start(out=y_d[tt * 128:(tt + 1) * 128, :], in_=ysb)

        # final 4 out-proj groups split so only the pair-3 contraction sits
        # in the serial tail: pairs 0-2 accumulate into SBUF during the last
        # section's exp-paced slack, pair 3 adds in after tht[3] lands
        yfin = {}

        def out_proj_early(tt):
            ysb = work.tile([128, C], F32, tag="ysbf", bufs=4, name="ysbf")
            for jc in range(JC):
                jsl = slice(jc * 512, (jc + 1) * 512)
                pool, tg = ((apsum, "acc") if (tt + jc) % 2 == 0
                            else (ppsum, "proj"))
                yps = pool.tile([128, 512], F32, tag=tg, name="yps")
                for pp in range(PAIRS - 1):
                    nc.tensor.matmul(
                        yps, lhsT=tht_sb[pp][:, tt * 128:(tt + 1) * 128],
                        rhs=wo_sb[pp][:, jsl],
                        start=(pp == 0), stop=(pp == PAIRS - 2))
                nc.vector.tensor_add(out=ysb[:, jsl], in0=yps,
                                     in1=bias_bc[:, jsl])
            yfin[tt] = ysb

        def out_proj_late(tt):
            ysb = yfin[tt]
            for jc in range(JC):
                jsl = slice(jc * 512, (jc + 1) * 512)
                pool, tg = ((ppsum, "proj") if (tt + jc) % 2 == 0
                            else (apsum, "acc"))
                yps = pool.tile([128, 512], F32, tag=tg, name="yps")
                nc.tensor.matmul(
                    yps, lhsT=tht_sb[PAIRS - 1][:, tt * 128:(tt + 1) * 128],
                    rhs=wo_sb[PAIRS - 1][:, jsl], start=True, stop=True)
                nc.vector.tensor_add(out=ysb[:, jsl], in0=yps, in1=ysb[:, jsl])
            # tail DMAs: split each row-block across engines/queues so the
            # final 2MB ships in parallel instead of serializing on one ring
            dengs = [nc.sync, nc.scalar, nc.gpsimd]
            for jc in range(JC):
                jsl = slice(jc * 512, (jc + 1) * 512)
                dengs[(2 * tt + jc) % 3].dma_start(
                    out=y_d[tt * 128:(tt + 1) * 128, jsl], in_=ysb[:, jsl])

        ring_base = 0
        for p in range(PAIRS):
            for qc in range(QC):
                qsl = slice(qc * 512, (qc + 1) * 512)

                def unit(kt, lh):
                    u = (ring_base + 2 * kt + lh) % RING
                    return slice(u * 512, (u + 1) * 512)

                # scores + exp: adjacent matmuls alternate PE row groups
                # (h0 rows 0-63, h1 rows 64-127) so they can overlap
                for kt in range(TT):
                    ps = spsum.tile([128, 1024], F32, tag="mm", name="sps")
                    for lh in range(2):
                        hsl = slice(lh * 64, (lh + 1) * 64)
                        nc.tensor.matmul(
                            ps[:, lh * 512:(lh + 1) * 512],
                            lhsT=kt_sb[p][hsl, kt * 128:(kt + 1) * 128],
                            rhs=qt_sb[p][hsl, qsl],
                            start=True, stop=True)
                    u0 = (ring_base + 2 * kt) % RING
                    if kt in DVE_KTS:
                        # exp on DVE: deg-4 poly then ^16, two custom passes
                        mid = work.tile([128, 1024], F32, tag="escr", name="escr")
                        nc.vector._custom_dve(
                            _EXPP4, out=mid, in0=ps, in1=c4_sb,
                            s0=_EXPC[0], s1=_EXPC[1], imm2=_EXPC[2])
                        nc.vector._custom_dve(
                            _SQ4, out=exps[:, u0 * 512:(u0 + 2) * 512], in0=mid)
                    else:
                        nc.scalar.activation(
                            out=exps[:, u0 * 512:(u0 + 2) * 512],
                            in_=ps, func=EXP, scale=16.0)
                    if p == 0 and qc == 0:
                        # V projection + remaining pair-0 K/Q chunks woven
                        # into the exp-paced score loop
                        v_chunk(kt)
                        for wname, fqc in p0_fillers.get(kt, []):
                            dst = qt_sb[0] if wname == "wq" else kt_sb[0]
                            project_chunk(0, dst, wname, fqc)

                # out-projection of the previous qc chunk, placed between
                # scores and PV: PV has exp-pacing slack to absorb it and the
                # next section's scores are not delayed behind it
                if p == PAIRS - 1 and qc >= 1:
                    for tt in range(4 * (qc - 1), 4 * qc):
                        out_proj_group(tt)
                    if qc == QC - 1:
                        for tt in range(4 * (QC - 1), 4 * QC):
                            out_proj_early(tt)
                # PV: both heads' accumulation chains interleaved so ring
                # units free in kt order and exp of the next section can
                # overwrite them while these chains drain. In the very last
                # section the heads run sequentially instead, so h0's
                # normalize chain overlaps h1's PV stream (shorter tail).
                ops = [apsum.tile([D + 1, 512], F32, tag="acc", name=f"ops{lh}")
                       for lh in range(2)]
                def normalize(lh):
                    # copy sums to partition 0 first: the custom-DVE fast
                    # reciprocal misreads partition-shifted inputs
                    ssb = work.tile([1, 512], F32, tag="ssb", name="ssb")
                    nc.vector.tensor_copy(out=ssb, in_=ops[lh][D:D + 1, :])
                    rsb = work.tile([1, 512], F32, tag="rsb", name="rsb")
                    nc.vector.reciprocal_approx_fast(out=rsb, in_=ssb)
                    rbc = work.tile([64, 512], F32, tag="rbc", name="rbc")
                    nc.gpsimd.partition_broadcast(rbc, rsb)
                    nc.vector.tensor_mul(
                        out=tht_sb[p][lh * 64:(lh + 1) * 64, qsl],
                        in0=ops[lh][0:D, :], in1=rbc)

                for lh in range(2):
                    for kt in range(TT):
                        nc.tensor.matmul(
                            ops[lh], lhsT=vaug[kt][:, 2 * p + lh, :],
                            rhs=exps[:, unit(kt, lh)],
                            start=(kt == 0), stop=(kt == TT - 1))
                    normalize(lh)
                ring_base = (ring_base + 2 * TT) % RING
                for fp, wname, fqc in fillers.get((p, qc), []):
                    dst = qt_sb[fp] if wname == "wq" else kt_sb[fp]
                    project_chunk(fp, dst, wname, fqc)

        for tt in range(4 * (QC - 1), 4 * QC):
            out_proj_late(tt)


def _build():
    nc = bacc.Bacc("TRN2", target_bir_lowering=False)
    xT_d = nc.dram_tensor("xT", [C, T], BF16, kind="ExternalInput")
    wq_d = nc.dram_tensor("wq", [C, 512], BF16, kind="ExternalInput")
    wk_d = nc.dram_tensor("wk", [C, 512], BF16, kind="ExternalInput")
    wv_d = nc.dram_tensor("wv", [C, 512], BF16, kind="ExternalInput")
    wo_d = nc.dram_tensor("wo", [512, C], BF16, kind="ExternalInput")
    bias_d = nc.dram_tensor("bias", [1, C], F32, kind="ExternalInput")
    y_d = nc.dram_tensor("y", [T, C], F32, kind="ExternalOutput")
    with tile.TileContext(nc) as tc:
        _emit(nc, tc, xT_d, wq_d, wk_d, wv_d, wo_d, bias_d, y_d)
    if not nc.is_finalized():
        nc.finalize()
    return nc


def get_nc():
    global _CACHED_NC
    if _CACHED_NC is None:
        _CACHED_NC = _build()
    return _CACHED_NC


def make_in_maps(x, w_qkv, w_out, b_out):
    bf = ml_dtypes.bfloat16
    x = np.asarray(x, dtype=np.float32)
    w_qkv = np.asarray(w_qkv, dtype=np.float32)
    w_out = np.asarray(w_out, dtype=np.float32)
    b_out = np.asarray(b_out, dtype=np.float32)
    in_maps = []
    for core in range(8):
        b, hg = core // 2, core % 2
        cs = slice(hg * 512, (hg + 1) * 512)
        bias = b_out if hg == 0 else np.zeros_like(b_out)
        in_maps.append({
            "xT": np.ascontiguousarray(x[b].T).astype(bf),
            # wq pre-scaled by the attention scale (1/8) / 16 so scores psum
            # holds m with exp(s/8) = exp(16m); 2^-7 is exact in bf16
            "wq": np.ascontiguousarray(w_qkv[:, 0 * C:][:, cs] * 0.0078125).astype(bf),
            "wk": np.ascontiguousarray(w_qkv[:, 1 * C:][:, cs]).astype(bf),
            "wv": np.ascontiguousarray(w_qkv[:, 2 * C:][:, cs]).astype(bf),
            "wo": np.ascontiguousarray(w_out[cs, :]).astype(bf),
            "bias": np.ascontiguousarray(bias.reshape(1, C), dtype=np.float32),
        })
    return in_maps


def _ensure_ntff_hook():
    """Register the axon NTFF profile hook if the container's antenv lacks
    axon_hooks (test/profiling use only; never needed for plain kernel())."""
    import sys
    import types
    try:
        from antenv import axon_hooks  # noqa: F401
    except ImportError:
        mod = types.ModuleType("antenv.axon_hooks")
        mod._hook = None

        def set_axon_ntff_profile_hook(hook, _m=mod):
            _m._hook = hook

        def get_axon_ntff_profile_hook(_m=mod):
            return _m._hook

        mod.set_axon_ntff_profile_hook = set_axon_ntff_profile_hook
        mod.get_axon_ntff_profile_hook = get_axon_ntff_profile_hook
        sys.modules["antenv.axon_hooks"] = mod
        import antenv
        antenv.axon_hooks = mod
    import antenv.axon_hooks as ah
    if ah.get_axon_ntff_profile_hook() is None:
        from trn_agent_boot.trn_boot import _ntff_profile_via_ctypes
        ah.set_axon_ntff_profile_hook(
            _ntff_profile_via_ctypes("/opt/axon/libaxon_pjrt.so"))


def kernel(x, w_qkv, w_out, b_out, _trace=False, _trace_kwargs=None):
    nc = get_nc()
    in_maps = make_in_maps(x, w_qkv, w_out, b_out)
    kwargs = {}
    if _trace:
        try:
            _ensure_ntff_hook()
        except Exception as e:
            print(f"NTFF hook setup failed ({e}); running without trace")
        else:
            kwargs.update(trace=True, **(_trace_kwargs or {}))
    res = run_bass_kernel_spmd(nc, in_maps, core_ids=list(range(8)), **kwargs)
    out = np.empty((B, T, C), dtype=np.float32)
    for b in range(B):
        out[b] = res.results[2 * b]["y"] + res.results[2 * b + 1]["y"]
    if _trace:
        return out, res
    return out

